# revision 1
# baseline (speedup 1.0000x reference)
"""DiGCNNet forward on 8 Trainium2 NeuronCores, data-parallel over batch.

Math (per batch b):
  adj = mean_t graph_sigs[b]                  # [30, 30]
  xw  = real[b] @ W                           # [30, 256]
  agg = adj^T @ xw + conv_bias                # [30, 256]
  h   = relu(agg)
  ns  = h @ pool_w + pool_b                   # [30]
  lg  = ns @ head_w^T + head_b                # [7]
  out = softmax(lg)

Device strategy per core (64 batches, processed in 16 groups of 4):
  - T-reduce as a PE matmul: ones^T(1/64) @ G with two batches stacked on the
    128 partitions (K=128), out PSUM [2, 900].
  - adj scatter: PSUM->SBUF copy (ACT) then SBUF->SBUF DMA [1,900] -> [30,30]
    diagonal blocks of a [121, 120] block-diagonal lhsT (row 120 = ones for
    the conv_bias contraction row).
  - xw: real loaded transposed via stride-1-partition DMA ([128(f), 4, 120(n)]),
    4 accumulating matmuls against pre-chunked W -> PSUM [120, 256].
  - agg: one block-diagonal matmul [121,120]^T @ [121,256] -> PSUM [120,256]
    (rhs row 120 = conv_bias).
  - relu on ACT, pool via one tensor_tensor_reduce (mult+add, init=pool_b).
  - head: constant block-diag head_w^T [120, 28] matmul -> logits [28, 1].
  - softmax tail on [28, 16] with 7-block partition sums done via tiny matmuls.
"""

from contextlib import ExitStack

import numpy as np

import concourse.bacc as bacc
import concourse.bass as bass
import concourse.tile as tile
from concourse import mybir
from concourse.bass_utils import run_bass_kernel_spmd

F32 = mybir.dt.float32
F32R = mybir.dt.float32r

B, T, N = 512, 64, 30
F_IN, D, C = 512, 256, 7
NCORES = 8
BL = B // NCORES        # 64 batches per core
GPB = 4                 # batches per group
NG = BL // GPB          # 16 groups
NN = N * N              # 900
NB = GPB * N            # 120 stacked node rows per group


def _build_nc():
    nc = bacc.Bacc(None, target_bir_lowering=False)

    gs = nc.dram_tensor("gs", (BL, T, N, N), F32, kind="ExternalInput")
    # real pre-transposed on host to [F_IN, BL*N] so chunk loads are
    # contiguous-innermost for the DMA engines.
    realt = nc.dram_tensor("realt", (F_IN, BL * N), F32, kind="ExternalInput")
    wt = nc.dram_tensor("wt", (128, 4, D), F32, kind="ExternalInput")
    cb = nc.dram_tensor("cb", (1, D), F32, kind="ExternalInput")
    pwb = nc.dram_tensor("pwb", (NB, D), F32, kind="ExternalInput")
    hwblk = nc.dram_tensor("hwblk", (NB, GPB * C), F32, kind="ExternalInput")
    hbb = nc.dram_tensor("hbb", (GPB * C, 1), F32, kind="ExternalInput")
    ones2 = nc.dram_tensor("ones2", (128, 2), F32, kind="ExternalInput")
    ones1 = nc.dram_tensor("ones1", (1, NB), F32, kind="ExternalInput")
    b7 = nc.dram_tensor("b7", (GPB * C, GPB), F32, kind="ExternalInput")
    b7t = nc.dram_tensor("b7t", (GPB, GPB * C), F32, kind="ExternalInput")
    out = nc.dram_tensor("out", (BL, C), F32, kind="ExternalOutput")

    with tile.TileContext(nc) as tc, ExitStack() as ctx:
        consts = ctx.enter_context(tc.tile_pool(name="consts", bufs=1))
        gt_pool = ctx.enter_context(tc.tile_pool(name="gt", bufs=8))
        adjs_pool = ctx.enter_context(tc.tile_pool(name="adjs", bufs=6))
        adjb_pool = ctx.enter_context(tc.tile_pool(name="adjb", bufs=16))
        xwb_pool = ctx.enter_context(tc.tile_pool(name="xwb", bufs=2))
        h_pool = ctx.enter_context(tc.tile_pool(name="h", bufs=2))
        scr_pool = ctx.enter_context(tc.tile_pool(name="scr", bufs=2))
        ns_pool = ctx.enter_context(tc.tile_pool(name="ns", bufs=2))
        tail_pool = ctx.enter_context(tc.tile_pool(name="tail", bufs=1))
        adjp_pool = ctx.enter_context(
            tc.tile_pool(name="adjp", bufs=2, space=bass.MemorySpace.PSUM)
        )
        xwp_pool = ctx.enter_context(
            tc.tile_pool(name="xwp", bufs=2, space=bass.MemorySpace.PSUM)
        )
        aggp_pool = ctx.enter_context(
            tc.tile_pool(name="aggp", bufs=1, space=bass.MemorySpace.PSUM)
        )
        smallp_pool = ctx.enter_context(
            tc.tile_pool(name="smallp", bufs=1, space=bass.MemorySpace.PSUM)
        )

        def load_const(dram, shape, dtype=F32):
            t = consts.tile(shape, dtype, tag=dram.name)
            src_ap = dram[:].bitcast(dtype) if dtype is not F32 else dram[:]
            nc.scalar.dma_start(t[:], src_ap)
            return t

        wt_sb = load_const(wt, [128, 4, D], F32R)
        cb_sb = load_const(cb, [1, D], F32R)
        pwb_sb = load_const(pwb, [NB, D])
        hw_sb = load_const(hwblk, [NB, GPB * C])
        hbb_sb = load_const(hbb, [GPB * C, 1])
        ones2_sb = load_const(ones2, [128, 2], F32R)
        ones1_sb = load_const(ones1, [1, NB], F32R)
        b7_sb = load_const(b7, [GPB * C, GPB])
        b7t_sb = load_const(b7t, [GPB, GPB * C])

        logits_all = consts.tile([GPB * C, NG], F32, tag="logits_all")

        # whole realt resident in SBUF: [128(f%128), 4(f//128), 1920(b*n)]
        rt_all = consts.tile([128, 4, BL * N], F32R, tag="rt_all")
        nc.sync.dma_start(
            rt_all[:], realt.rearrange("(c p) m -> p c m", p=128).bitcast(F32R)
        )

        # ---- phase A: T-reduce all groups into persistent block-diag tiles
        adjb_tiles = []
        for g in range(NG):
            adjb_t = adjb_pool.tile([NB, NB], F32R, tag="adjb")
            nc.vector.memset(adjb_t[:].bitcast(F32), 0.0)
            adjb_tiles.append(adjb_t)

        for g in range(NG):
            b0 = g * GPB
            adjs_tiles = []
            for p2 in range(2):
                bb = b0 + 2 * p2
                gtile = gt_pool.tile([128, NN], F32R, tag="gt")
                nc.sync.dma_start(
                    gtile[:],
                    gs[bb : bb + 2].rearrange("b t i j -> (b t) (i j)").bitcast(F32R),
                )
                adjp_t = adjp_pool.tile([2, NN], F32, tag="adjp")
                nc.tensor.matmul(
                    adjp_t[:, 0:512], ones2_sb[:], gtile[:, 0:512],
                    start=True, stop=True,
                )
                nc.tensor.matmul(
                    adjp_t[:, 512:NN], ones2_sb[:], gtile[:, 512:NN],
                    start=True, stop=True,
                )
                adjs_t = adjs_pool.tile([2, NN], F32, tag="adjs")
                if p2 == 0:
                    nc.scalar.copy(adjs_t[:], adjp_t[:])
                else:
                    nc.vector.tensor_copy(adjs_t[:], adjp_t[:])
                adjs_tiles.append(adjs_t)
            for k in range(GPB):
                nc.gpsimd.dma_start(
                    adjb_tiles[g][k * N : (k + 1) * N, k * N : (k + 1) * N],
                    adjs_tiles[k // 2][k % 2 : k % 2 + 1, :].bitcast(F32R),
                )

        # ---- phase B: xw -> agg -> relu -> pool -> head per group
        for g in range(NG):
            b0 = g * GPB
            xwp_t = xwp_pool.tile([NB, D], F32, tag="xwp")
            for c4 in range(4):
                nc.tensor.matmul(
                    xwp_t[:], rt_all[:, c4, b0 * N : (b0 + GPB) * N],
                    wt_sb[:, c4, :], start=(c4 == 0), stop=(c4 == 3),
                )
            xwb_t = xwb_pool.tile([NB, D], F32R, tag="xwb")
            nc.vector.tensor_copy(xwb_t[:], xwp_t[:])

            aggp_t = aggp_pool.tile([NB, D], F32, tag="aggp")
            nc.tensor.matmul(
                aggp_t[:], adjb_tiles[g][:], xwb_t[:], start=True, stop=False,
            )
            nc.tensor.matmul(
                aggp_t[:], ones1_sb[:], cb_sb[:], start=False, stop=True,
            )

            h_t = h_pool.tile([NB, D], F32, tag="h")
            nc.scalar.activation(h_t[:], aggp_t[:], mybir.ActivationFunctionType.Relu)
            scr_t = scr_pool.tile([NB, D], F32, tag="scr")
            ns_t = ns_pool.tile([NB, 1], F32, tag="ns")
            nc.vector.tensor_mul(scr_t[:], h_t[:], pwb_sb[:])
            nc.vector.reduce_sum(ns_t[:], scr_t[:], axis=mybir.AxisListType.X)

            lg_t = smallp_pool.tile([GPB * C, 1], F32, tag="small")
            nc.tensor.matmul(lg_t[:], hw_sb[:], ns_t[:], start=True, stop=True)
            nc.vector.tensor_add(logits_all[:, g : g + 1], lg_t[:], hbb_sb[:])

        # ---- softmax over the 7 classes (partition sub-blocks of 7)
        e_t = tail_pool.tile([GPB * C, NG], F32, tag="e")
        nc.scalar.activation(e_t[:], logits_all[:], mybir.ActivationFunctionType.Exp)
        sum_p = smallp_pool.tile([GPB, NG], F32, tag="small")
        nc.tensor.matmul(sum_p[:], b7_sb[:], e_t[:], start=True, stop=True)
        ssb_t = tail_pool.tile([GPB, NG], F32, tag="ssb")
        nc.vector.tensor_copy(ssb_t[:], sum_p[:])
        bcast_p = smallp_pool.tile([GPB * C, NG], F32, tag="small")
        nc.tensor.matmul(bcast_p[:], b7t_sb[:], ssb_t[:], start=True, stop=True)
        rs_t = tail_pool.tile([GPB * C, NG], F32, tag="rs")
        nc.vector.reciprocal(rs_t[:], bcast_p[:])
        res_t = tail_pool.tile([GPB * C, NG], F32, tag="res")
        nc.vector.tensor_mul(res_t[:], e_t[:], rs_t[:])
        nc.scalar.dma_start(out.rearrange("(g bi) c -> (bi c) g", bi=GPB), res_t[:])

    nc.compile()
    return nc


_NC_CACHE = None


def _get_nc():
    global _NC_CACHE
    if _NC_CACHE is None:
        _NC_CACHE = _build_nc()
    return _NC_CACHE


def _f32c(x):
    return np.ascontiguousarray(np.asarray(x, dtype=np.float32))


def _prepare_in_maps(real, graph_sigs, W, conv_bias, pool_w, pool_b, head_w, head_b):
    real = _f32c(real)
    graph_sigs = _f32c(graph_sigs)
    W = _f32c(W)

    wt = np.ascontiguousarray(
        _f32c(W).reshape(4, 128, D).transpose(1, 0, 2)
    )  # [128(f%128), 4(f//128), 256]
    cb = _f32c(conv_bias).reshape(1, D)
    pwb = np.ascontiguousarray(np.broadcast_to(_f32c(pool_w), (NB, D)))
    hw_t = _f32c(head_w).T  # [30, 7]
    hwblk = np.zeros((NB, GPB * C), dtype=np.float32)
    for k in range(GPB):
        hwblk[k * N : (k + 1) * N, k * C : (k + 1) * C] = hw_t
    # pool_b shifts every node score by a constant; fold it into the head
    # bias: logits[c] += pool_b * sum_j head_w[c, j]
    hb_eff = _f32c(head_b) + np.float32(np.asarray(pool_b)) * _f32c(head_w).sum(axis=1)
    hbb = np.tile(hb_eff, GPB).reshape(GPB * C, 1)
    ones2 = np.zeros((128, 2), dtype=np.float32)
    ones2[0:64, 0] = 1.0 / T
    ones2[64:128, 1] = 1.0 / T
    b7 = np.zeros((GPB * C, GPB), dtype=np.float32)
    for k in range(GPB):
        b7[k * C : (k + 1) * C, k] = 1.0
    b7t = np.ascontiguousarray(b7.T)
    ones1 = np.ones((1, NB), dtype=np.float32)

    consts = {
        "wt": wt, "cb": cb, "pwb": pwb, "hwblk": hwblk,
        "hbb": hbb, "ones2": ones2, "ones1": ones1, "b7": b7, "b7t": b7t,
    }
    in_maps = []
    for c in range(NCORES):
        s = slice(c * BL, (c + 1) * BL)
        in_maps.append(
            {
                "gs": np.ascontiguousarray(graph_sigs[s]),
                "realt": np.ascontiguousarray(
                    real[s].transpose(2, 0, 1).reshape(F_IN, BL * N)
                ),
                **consts,
            }
        )
    return in_maps


def kernel(real, imag, graph_sigs, W, conv_bias, pool_w, pool_b, head_w, head_b):
    del imag  # unused by the forward pass
    in_maps = _prepare_in_maps(
        real, graph_sigs, W, conv_bias, pool_w, pool_b, head_w, head_b
    )
    nc = _get_nc()
    res = run_bass_kernel_spmd(nc, in_maps, core_ids=list(range(NCORES)))
    return np.concatenate([res.results[c]["out"] for c in range(NCORES)], axis=0)

